# revision 2
# baseline (speedup 1.0000x reference)
"""Trainium2 Bass kernel for nn_Alpha2Assoc: 3-layer alpha compositing
with exclusive cumprod over D=32 planes.

Math per pixel (d = plane index, vectors over d):
    a1 = alpha                      (clip at 1e-4 dropped: ~1e-4 rel effect)
    vis_l = excl_cumprod(1 - a_l);  out_l = vis_l * occ_{l-1};
    occ_l = 1 - vis_l;              a_{l+1} = a_l * occ_l
Output: stack([out_1, out_2, out_3], axis=2) -> [B, D, 3, H, W]

Implementation: cumprod in log space.  u = ln(1 - a) on ScalarE (free
affine absorbs the 1-x), exclusive cumsum via TensorE matmul with a
block-diagonal strictly-lower-triangular 0/1 matrix (4 b-groups x 32
planes packed into K=128 partitions), vis = exp(.) on ScalarE from
PSUM.  The a-recursion is kept as s2 = (vis1-1)*a1 = -a2 and
s3 = (vis2-1)*s2 = +a3 (single fused scalar_tensor_tensor each); the
sign flip is absorbed by Ln's scale argument.

Sharding: pure data parallel over H: core c takes h in [c*64,(c+1)*64).
"""

import numpy as np

import concourse.bass as bass
import concourse.tile as tile
from concourse import bacc, mybir
from concourse._compat import with_exitstack
from concourse.bass_utils import run_bass_kernel_spmd

B, D, H, W = 4, 32, 512, 512
N_CORES = 8
H_SH = H // N_CORES              # 64 rows per core
N_PIX = B * H_SH * W // B        # pixel columns per core per (b,d) row = 32768
assert N_PIX == H_SH * W
P = B * D                        # 128 partitions = (b, d)
TILE_N = 2048
N_TILES = N_PIX // TILE_N
MM_CHUNK = 512                   # f32 moving-operand max per matmul

F32 = mybir.dt.float32
AF = mybir.ActivationFunctionType
OP = mybir.AluOpType

_COMPILED = {}


def _tri_matrix() -> np.ndarray:
    """lhsT[k, m] = 1 iff same b-block and d_k < d_m  (exclusive cumsum)."""
    k = np.arange(P)
    m = np.arange(P)
    same_b = (k[:, None] // D) == (m[None, :] // D)
    lower = (k[:, None] % D) < (m[None, :] % D)
    return (same_b & lower).astype(np.float32)


@with_exitstack
def _alpha_kernel(ctx, tc, out_aps, in_ap, tri_ap):
    nc = tc.nc
    const_pool = ctx.enter_context(tc.tile_pool(name="const", bufs=1))
    a_pool = ctx.enter_context(tc.tile_pool(name="a", bufs=3))
    u_pool = ctx.enter_context(tc.tile_pool(name="u", bufs=4))
    vis_pool = ctx.enter_context(tc.tile_pool(name="vis", bufs=4))
    s_pool = ctx.enter_context(tc.tile_pool(name="s", bufs=3))
    occ_pool = ctx.enter_context(tc.tile_pool(name="occ", bufs=3))
    o_pool = ctx.enter_context(tc.tile_pool(name="o", bufs=3))
    psum_pool = ctx.enter_context(tc.tile_pool(name="ps", bufs=2, space="PSUM"))

    tri = const_pool.tile([P, P], F32)
    nc.sync.dma_start(tri[:], tri_ap[:, :])

    def cumsum_exp(u, vis):
        ps = psum_pool.tile([P, TILE_N], F32, tag="ps")
        for j in range(TILE_N // MM_CHUNK):
            nc.tensor.matmul(
                ps[:, bass.ts(j, MM_CHUNK)],
                tri[:],
                u[:, bass.ts(j, MM_CHUNK)],
                start=True,
                stop=True,
            )
        nc.scalar.activation(vis[:], ps[:], AF.Exp)

    for i in range(N_TILES):
        sl = bass.ts(i, TILE_N)
        a1 = a_pool.tile([P, TILE_N], F32, tag="a")
        nc.sync.dma_start(a1[:], in_ap[:, sl])

        # layer 1: vis1 = exp(cumsum(ln(1 - a1)))
        u1 = u_pool.tile([P, TILE_N], F32, tag="u")
        nc.scalar.activation(u1[:], a1[:], AF.Ln, bias=1.0, scale=-1.0)
        vis1 = vis_pool.tile([P, TILE_N], F32, tag="vis")
        cumsum_exp(u1, vis1)
        nc.sync.dma_start(out_aps[0][:, sl], vis1[:])   # out1 = vis1 (occ0=1)

        # s2 = (vis1 - 1) * a1 = -a2
        s2 = s_pool.tile([P, TILE_N], F32, tag="s")
        nc.vector.scalar_tensor_tensor(
            s2[:], vis1[:], 1.0, a1[:], OP.subtract, OP.mult
        )

        # layer 2: u2 = ln(1 + s2) = ln(1 - a2)
        u2 = u_pool.tile([P, TILE_N], F32, tag="u")
        nc.scalar.activation(u2[:], s2[:], AF.Ln, bias=1.0, scale=1.0)
        vis2 = vis_pool.tile([P, TILE_N], F32, tag="vis")
        cumsum_exp(u2, vis2)
        occ1 = occ_pool.tile([P, TILE_N], F32, tag="occ")
        nc.vector.tensor_scalar(occ1[:], vis1[:], -1.0, 1.0, OP.mult, OP.add)
        o2 = o_pool.tile([P, TILE_N], F32, tag="o")
        nc.vector.tensor_mul(o2[:], vis2[:], occ1[:])
        nc.sync.dma_start(out_aps[1][:, sl], o2[:])

        # s3 = (vis2 - 1) * s2 = +a3
        s3 = s_pool.tile([P, TILE_N], F32, tag="s")
        nc.vector.scalar_tensor_tensor(
            s3[:], vis2[:], 1.0, s2[:], OP.subtract, OP.mult
        )

        # layer 3: u3 = ln(1 - s3) = ln(1 - a3)
        u3 = u_pool.tile([P, TILE_N], F32, tag="u")
        nc.scalar.activation(u3[:], s3[:], AF.Ln, bias=1.0, scale=-1.0)
        vis3 = vis_pool.tile([P, TILE_N], F32, tag="vis")
        cumsum_exp(u3, vis3)
        occ2 = occ_pool.tile([P, TILE_N], F32, tag="occ")
        nc.vector.tensor_scalar(occ2[:], vis2[:], -1.0, 1.0, OP.mult, OP.add)
        o3 = o_pool.tile([P, TILE_N], F32, tag="o")
        nc.vector.tensor_mul(o3[:], vis3[:], occ2[:])
        nc.sync.dma_start(out_aps[2][:, sl], o3[:])


def _build():
    nc = bacc.Bacc("TRN2", target_bir_lowering=False, debug=False,
                   num_devices=N_CORES)
    in_ap = nc.dram_tensor("alpha", [P, N_PIX], F32, kind="ExternalInput").ap()
    tri_ap = nc.dram_tensor("tri", [P, P], F32, kind="ExternalInput").ap()
    out_aps = [
        nc.dram_tensor(f"out{c}", [P, N_PIX], F32, kind="ExternalOutput").ap()
        for c in range(3)
    ]
    with tile.TileContext(nc) as tc:
        _alpha_kernel(tc, out_aps, in_ap, tri_ap)
    nc.compile()
    return nc


def _get_nc():
    if "nc" not in _COMPILED:
        _COMPILED["nc"] = _build()
    return _COMPILED["nc"]


def _run(alpha_imgs: np.ndarray, trace: bool = False):
    nc = _get_nc()
    tri = _tri_matrix()
    a = np.asarray(alpha_imgs, dtype=np.float32)
    in_maps = []
    for c in range(N_CORES):
        shard = np.ascontiguousarray(
            a[:, :, 0, c * H_SH:(c + 1) * H_SH, :]
        ).reshape(P, N_PIX)
        in_maps.append({"alpha": shard, "tri": tri})
    res = run_bass_kernel_spmd(
        nc, in_maps, core_ids=list(range(N_CORES)), trace=trace
    )
    out = np.empty((B, D, 3, H, W), dtype=np.float32)
    for c in range(N_CORES):
        r = res.results[c]
        oc = np.stack(
            [r["out0"], r["out1"], r["out2"]], axis=1
        )  # [P, 3, N_PIX]
        oc = oc.reshape(B, D, 3, H_SH, W)
        out[:, :, :, c * H_SH:(c + 1) * H_SH, :] = oc
    return out, res


def kernel(alpha_imgs: np.ndarray) -> np.ndarray:
    out, _ = _run(alpha_imgs, trace=False)
    return out


# revision 5
# speedup vs baseline: 1.5536x; 1.5536x over previous
"""Trainium2 Bass kernel for nn_Alpha2Assoc: 3-layer alpha compositing
with exclusive cumprod over D=32 planes.

Math per pixel (d = plane index, vectors over d):
    a1 = alpha                      (clip at 1e-4 dropped: ~1e-4 rel effect)
    vis_l = excl_cumprod(1 - a_l);  out_l = vis_l * occ_{l-1};
    occ_l = 1 - vis_l;              a_{l+1} = a_l * occ_l
Output: stack([out_1, out_2, out_3], axis=2) -> [B, D, 3, H, W]

Implementation: cumprod in log space.  u = ln(1 - a) on ScalarE (free
affine absorbs the 1-x), exclusive cumsum via TensorE matmul with a
block-diagonal strictly-lower-triangular 0/1 matrix (4 b-groups x 32
planes packed into K=128 partitions), vis = exp(.) on ScalarE from
PSUM.  The a-recursion is kept as s2 = (vis1-1)*a1 = -a2 and
s3 = (vis2-1)*s2 = +a3 (single fused scalar_tensor_tensor each); the
sign flip is absorbed by Ln's scale argument.

Sharding: pure data parallel over H: core c takes h in [c*64,(c+1)*64).
"""

import numpy as np

import concourse.bass as bass
import concourse.tile as tile
from concourse import bacc, mybir
from concourse._compat import with_exitstack
from concourse.bass_utils import run_bass_kernel_spmd

# --- Pin Ln/Exp to the one table set containing both -------------------
# The act-table chooser picks a preferred set per function; Ln and Exp
# alternate per layer, which thrashes ACT_TABLE_LOAD (~1.5us each).
# Keep set indices stable but remove Ln/Exp membership from every set
# except natural_log_exp_and_others so one load serves the whole kernel.
_orig_get_activation_tables = bacc.get_activation_tables


def _pinned_get_activation_tables(arch):
    tables = _orig_get_activation_tables(arch)
    keep = {mybir.ActivationFunctionType.Ln, mybir.ActivationFunctionType.Exp}
    return {
        name: (fns if name == "natural_log_exp_and_others" else fns - keep)
        for name, fns in tables.items()
    }


bacc.get_activation_tables = _pinned_get_activation_tables

B, D, H, W = 4, 32, 512, 512
N_CORES = 8
H_SH = H // N_CORES              # 64 rows per core
N_PIX = B * H_SH * W // B        # pixel columns per core per (b,d) row = 32768
assert N_PIX == H_SH * W
P = B * D                        # 128 partitions = (b, d)
TILE_N = 2048
N_TILES = N_PIX // TILE_N
MM_CHUNK = 512                   # f32 moving-operand max per matmul

F32 = mybir.dt.float32
F32R = mybir.dt.float32r
AF = mybir.ActivationFunctionType
OP = mybir.AluOpType

_COMPILED = {}


def _tri_matrix() -> np.ndarray:
    """lhsT[k, m] = 1 iff same b-block and d_k < d_m  (exclusive cumsum)."""
    k = np.arange(P)
    m = np.arange(P)
    same_b = (k[:, None] // D) == (m[None, :] // D)
    lower = (k[:, None] % D) < (m[None, :] % D)
    return (same_b & lower).astype(np.float32)


@with_exitstack
def _alpha_kernel(ctx, tc, out_aps, in_ap, tri_ap):
    nc = tc.nc
    const_pool = ctx.enter_context(tc.tile_pool(name="const", bufs=1))
    a_pool = ctx.enter_context(tc.tile_pool(name="a", bufs=3))
    u_pool = ctx.enter_context(tc.tile_pool(name="u", bufs=4))
    vis_pool = ctx.enter_context(tc.tile_pool(name="vis", bufs=4))
    s_pool = ctx.enter_context(tc.tile_pool(name="s", bufs=3))
    occ_pool = ctx.enter_context(tc.tile_pool(name="occ", bufs=3))
    o_pool = ctx.enter_context(tc.tile_pool(name="o", bufs=3))
    psum_pool = ctx.enter_context(tc.tile_pool(name="ps", bufs=2, space="PSUM"))

    tri = const_pool.tile([P, P], F32R)
    nc.sync.dma_start(tri[:], tri_ap[:, :])

    def cumsum_exp(u, vis):
        # float32r: single-pass fp32 matmul (1 cyc/row vs 4 for fp32);
        # tri is 0/1 so only u loses mantissa bits (~2^-11 rel, fine here).
        ps = psum_pool.tile([P, TILE_N], F32, tag="ps")
        for j in range(TILE_N // MM_CHUNK):
            nc.tensor.matmul(
                ps[:, bass.ts(j, MM_CHUNK)],
                tri[:],
                u[:, bass.ts(j, MM_CHUNK)],
                start=True,
                stop=True,
            )
        nc.scalar.activation(vis[:], ps[:], AF.Exp)

    for i in range(N_TILES):
        sl = bass.ts(i, TILE_N)
        a1 = a_pool.tile([P, TILE_N], F32, tag="a")
        nc.sync.dma_start(a1[:], in_ap[:, sl])

        # layer 1: vis1 = exp(cumsum(ln(1 - a1)))
        u1 = u_pool.tile([P, TILE_N], F32R, tag="u")
        nc.scalar.activation(u1[:], a1[:], AF.Ln, bias=1.0, scale=-1.0)
        vis1 = vis_pool.tile([P, TILE_N], F32, tag="vis")
        cumsum_exp(u1, vis1)
        nc.sync.dma_start(out_aps[0][:, sl], vis1[:])   # out1 = vis1 (occ0=1)

        # s2 = (vis1 - 1) * a1 = -a2
        s2 = s_pool.tile([P, TILE_N], F32, tag="s")
        nc.vector.scalar_tensor_tensor(
            s2[:], vis1[:], 1.0, a1[:], OP.subtract, OP.mult
        )

        # layer 2: u2 = ln(1 + s2) = ln(1 - a2)
        u2 = u_pool.tile([P, TILE_N], F32R, tag="u")
        nc.scalar.activation(u2[:], s2[:], AF.Ln, bias=1.0, scale=1.0)
        vis2 = vis_pool.tile([P, TILE_N], F32, tag="vis")
        cumsum_exp(u2, vis2)
        occ1 = occ_pool.tile([P, TILE_N], F32, tag="occ")
        nc.vector.tensor_scalar(occ1[:], vis1[:], -1.0, 1.0, OP.mult, OP.add)
        o2 = o_pool.tile([P, TILE_N], F32, tag="o")
        nc.vector.tensor_mul(o2[:], vis2[:], occ1[:])
        nc.sync.dma_start(out_aps[1][:, sl], o2[:])

        # s3 = (vis2 - 1) * s2 = +a3
        s3 = s_pool.tile([P, TILE_N], F32, tag="s")
        nc.vector.scalar_tensor_tensor(
            s3[:], vis2[:], 1.0, s2[:], OP.subtract, OP.mult
        )

        # layer 3: u3 = ln(1 - s3) = ln(1 - a3)
        u3 = u_pool.tile([P, TILE_N], F32R, tag="u")
        nc.scalar.activation(u3[:], s3[:], AF.Ln, bias=1.0, scale=-1.0)
        vis3 = vis_pool.tile([P, TILE_N], F32, tag="vis")
        cumsum_exp(u3, vis3)
        occ2 = occ_pool.tile([P, TILE_N], F32, tag="occ")
        nc.vector.tensor_scalar(occ2[:], vis2[:], -1.0, 1.0, OP.mult, OP.add)
        o3 = o_pool.tile([P, TILE_N], F32, tag="o")
        nc.vector.tensor_mul(o3[:], vis3[:], occ2[:])
        nc.sync.dma_start(out_aps[2][:, sl], o3[:])


def _build():
    nc = bacc.Bacc("TRN2", target_bir_lowering=False, debug=False,
                   num_devices=N_CORES)
    in_ap = nc.dram_tensor("alpha", [P, N_PIX], F32, kind="ExternalInput").ap()
    tri_ap = nc.dram_tensor("tri", [P, P], F32R, kind="ExternalInput").ap()
    out_aps = [
        nc.dram_tensor(f"out{c}", [P, N_PIX], F32, kind="ExternalOutput").ap()
        for c in range(3)
    ]
    with tile.TileContext(nc) as tc:
        _alpha_kernel(tc, out_aps, in_ap, tri_ap)
    nc.compile()
    return nc


def _get_nc():
    if "nc" not in _COMPILED:
        _COMPILED["nc"] = _build()
    return _COMPILED["nc"]


def _run(alpha_imgs: np.ndarray, trace: bool = False):
    nc = _get_nc()
    tri = _tri_matrix()
    a = np.asarray(alpha_imgs, dtype=np.float32)
    in_maps = []
    for c in range(N_CORES):
        shard = np.ascontiguousarray(
            a[:, :, 0, c * H_SH:(c + 1) * H_SH, :]
        ).reshape(P, N_PIX)
        in_maps.append({"alpha": shard, "tri": tri})
    res = run_bass_kernel_spmd(
        nc, in_maps, core_ids=list(range(N_CORES)), trace=trace
    )
    out = np.empty((B, D, 3, H, W), dtype=np.float32)
    for c in range(N_CORES):
        r = res.results[c]
        oc = np.stack(
            [r["out0"], r["out1"], r["out2"]], axis=1
        )  # [P, 3, N_PIX]
        oc = oc.reshape(B, D, 3, H_SH, W)
        out[:, :, :, c * H_SH:(c + 1) * H_SH, :] = oc
    return out, res


def kernel(alpha_imgs: np.ndarray) -> np.ndarray:
    out, _ = _run(alpha_imgs, trace=False)
    return out


# revision 6
# speedup vs baseline: 1.6104x; 1.0366x over previous
"""Trainium2 Bass kernel for nn_Alpha2Assoc: 3-layer alpha compositing
with exclusive cumprod over D=32 planes.

Math per pixel (d = plane index, vectors over d):
    a1 = alpha                      (clip at 1e-4 dropped: ~1e-4 rel effect)
    vis_l = excl_cumprod(1 - a_l);  out_l = vis_l * occ_{l-1};
    occ_l = 1 - vis_l;              a_{l+1} = a_l * occ_l
Output: stack([out_1, out_2, out_3], axis=2) -> [B, D, 3, H, W]

Implementation: cumprod in log space.  u = ln(1 - a) on ScalarE (free
affine absorbs the 1-x), exclusive cumsum via TensorE matmul with a
block-diagonal strictly-lower-triangular 0/1 matrix (4 b-groups x 32
planes packed into K=128 partitions), vis = exp(.) on ScalarE from
PSUM.  The a-recursion is kept as s2 = (vis1-1)*a1 = -a2 and
s3 = (vis2-1)*s2 = +a3 (single fused scalar_tensor_tensor each); the
sign flip is absorbed by Ln's scale argument.

Sharding: pure data parallel over H: core c takes h in [c*64,(c+1)*64).
"""

import numpy as np

import concourse.bass as bass
import concourse.tile as tile
from concourse import bacc, mybir
from concourse._compat import with_exitstack
from concourse.bass_utils import run_bass_kernel_spmd

# --- Pin Ln/Exp to the one table set containing both -------------------
# The act-table chooser picks a preferred set per function; Ln and Exp
# alternate per layer, which thrashes ACT_TABLE_LOAD (~1.5us each).
# Keep set indices stable but remove Ln/Exp membership from every set
# except natural_log_exp_and_others so one load serves the whole kernel.
_orig_get_activation_tables = bacc.get_activation_tables


def _pinned_get_activation_tables(arch):
    tables = _orig_get_activation_tables(arch)
    keep = {mybir.ActivationFunctionType.Ln, mybir.ActivationFunctionType.Exp}
    return {
        name: (fns if name == "natural_log_exp_and_others" else fns - keep)
        for name, fns in tables.items()
    }


bacc.get_activation_tables = _pinned_get_activation_tables

B, D, H, W = 4, 32, 512, 512
N_CORES = 8
H_SH = H // N_CORES              # 64 rows per core
N_PIX = B * H_SH * W // B        # pixel columns per core per (b,d) row = 32768
assert N_PIX == H_SH * W
P = B * D                        # 128 partitions = (b, d)
TILE_N = 2048
N_TILES = N_PIX // TILE_N
MM_CHUNK = 512                   # f32 moving-operand max per matmul

F32 = mybir.dt.float32
F32R = mybir.dt.float32r
AF = mybir.ActivationFunctionType
OP = mybir.AluOpType

_COMPILED = {}


def _tri_matrix() -> np.ndarray:
    """lhsT[k, m] = 1 iff same b-block and d_k < d_m  (exclusive cumsum)."""
    k = np.arange(P)
    m = np.arange(P)
    same_b = (k[:, None] // D) == (m[None, :] // D)
    lower = (k[:, None] % D) < (m[None, :] % D)
    return (same_b & lower).astype(np.float32)


@with_exitstack
def _alpha_kernel(ctx, tc, out_aps, in_ap, tri_ap):
    nc = tc.nc
    const_pool = ctx.enter_context(tc.tile_pool(name="const", bufs=1))
    a_pool = ctx.enter_context(tc.tile_pool(name="a", bufs=3))
    u_pool = ctx.enter_context(tc.tile_pool(name="u", bufs=5))
    vis_pool = ctx.enter_context(tc.tile_pool(name="vis", bufs=5))
    s_pool = ctx.enter_context(tc.tile_pool(name="s", bufs=3))
    occ_pool = ctx.enter_context(tc.tile_pool(name="occ", bufs=3))
    o_pool = ctx.enter_context(tc.tile_pool(name="o", bufs=4))
    psum_pool = ctx.enter_context(tc.tile_pool(name="ps", bufs=2, space="PSUM"))

    tri = const_pool.tile([P, P], F32R)
    nc.sync.dma_start(tri[:], tri_ap[:, :])

    def cumsum_exp(u, vis):
        # float32r: single-pass fp32 matmul (1 cyc/row vs 4 for fp32);
        # tri is 0/1 so only u loses mantissa bits (~2^-11 rel, fine here).
        ps = psum_pool.tile([P, TILE_N], F32, tag="ps")
        for j in range(TILE_N // MM_CHUNK):
            nc.tensor.matmul(
                ps[:, bass.ts(j, MM_CHUNK)],
                tri[:],
                u[:, bass.ts(j, MM_CHUNK)],
                start=True,
                stop=True,
            )
        nc.scalar.activation(vis[:], ps[:], AF.Exp)

    for i in range(N_TILES):
        sl = bass.ts(i, TILE_N)
        a1 = a_pool.tile([P, TILE_N], F32, tag="a")
        nc.sync.dma_start(a1[:], in_ap[:, sl])

        # layer 1: vis1 = exp(cumsum(ln(1 - a1)))
        u1 = u_pool.tile([P, TILE_N], F32R, tag="u")
        nc.scalar.activation(u1[:], a1[:], AF.Ln, bias=1.0, scale=-1.0)
        vis1 = vis_pool.tile([P, TILE_N], F32, tag="vis")
        cumsum_exp(u1, vis1)
        nc.sync.dma_start(out_aps[0][:, sl], vis1[:])   # out1 = vis1 (occ0=1)

        # s2 = (vis1 - 1) * a1 = -a2
        s2 = s_pool.tile([P, TILE_N], F32, tag="s")
        nc.vector.scalar_tensor_tensor(
            s2[:], vis1[:], 1.0, a1[:], OP.subtract, OP.mult
        )

        # layer 2: u2 = ln(1 + s2) = ln(1 - a2)
        u2 = u_pool.tile([P, TILE_N], F32R, tag="u")
        nc.scalar.activation(u2[:], s2[:], AF.Ln, bias=1.0, scale=1.0)
        vis2 = vis_pool.tile([P, TILE_N], F32, tag="vis")
        cumsum_exp(u2, vis2)
        occ1 = occ_pool.tile([P, TILE_N], F32, tag="occ")
        nc.gpsimd.tensor_scalar(occ1[:], vis1[:], -1.0, 1.0, OP.mult, OP.add)
        o2 = o_pool.tile([P, TILE_N], F32, tag="o")
        nc.vector.tensor_mul(o2[:], vis2[:], occ1[:])
        nc.sync.dma_start(out_aps[1][:, sl], o2[:])

        # s3 = (vis2 - 1) * s2 = +a3
        s3 = s_pool.tile([P, TILE_N], F32, tag="s")
        nc.vector.scalar_tensor_tensor(
            s3[:], vis2[:], 1.0, s2[:], OP.subtract, OP.mult
        )

        # layer 3: u3 = ln(1 - s3) = ln(1 - a3)
        u3 = u_pool.tile([P, TILE_N], F32R, tag="u")
        nc.scalar.activation(u3[:], s3[:], AF.Ln, bias=1.0, scale=-1.0)
        vis3 = vis_pool.tile([P, TILE_N], F32, tag="vis")
        cumsum_exp(u3, vis3)
        occ2 = occ_pool.tile([P, TILE_N], F32, tag="occ")
        nc.gpsimd.tensor_scalar(occ2[:], vis2[:], -1.0, 1.0, OP.mult, OP.add)
        o3 = o_pool.tile([P, TILE_N], F32, tag="o")
        nc.vector.tensor_mul(o3[:], vis3[:], occ2[:])
        nc.sync.dma_start(out_aps[2][:, sl], o3[:])


def _build():
    nc = bacc.Bacc("TRN2", target_bir_lowering=False, debug=False,
                   num_devices=N_CORES)
    in_ap = nc.dram_tensor("alpha", [P, N_PIX], F32, kind="ExternalInput").ap()
    tri_ap = nc.dram_tensor("tri", [P, P], F32R, kind="ExternalInput").ap()
    out_aps = [
        nc.dram_tensor(f"out{c}", [P, N_PIX], F32, kind="ExternalOutput").ap()
        for c in range(3)
    ]
    with tile.TileContext(nc) as tc:
        _alpha_kernel(tc, out_aps, in_ap, tri_ap)
    nc.compile()
    return nc


def _get_nc():
    if "nc" not in _COMPILED:
        _COMPILED["nc"] = _build()
    return _COMPILED["nc"]


def _run(alpha_imgs: np.ndarray, trace: bool = False):
    nc = _get_nc()
    tri = _tri_matrix()
    a = np.asarray(alpha_imgs, dtype=np.float32)
    in_maps = []
    for c in range(N_CORES):
        shard = np.ascontiguousarray(
            a[:, :, 0, c * H_SH:(c + 1) * H_SH, :]
        ).reshape(P, N_PIX)
        in_maps.append({"alpha": shard, "tri": tri})
    res = run_bass_kernel_spmd(
        nc, in_maps, core_ids=list(range(N_CORES)), trace=trace
    )
    out = np.empty((B, D, 3, H, W), dtype=np.float32)
    for c in range(N_CORES):
        r = res.results[c]
        oc = np.stack(
            [r["out0"], r["out1"], r["out2"]], axis=1
        )  # [P, 3, N_PIX]
        oc = oc.reshape(B, D, 3, H_SH, W)
        out[:, :, :, c * H_SH:(c + 1) * H_SH, :] = oc
    return out, res


def kernel(alpha_imgs: np.ndarray) -> np.ndarray:
    out, _ = _run(alpha_imgs, trace=False)
    return out
